# revision 23
# baseline (speedup 1.0000x reference)
"""Single-head attention with QKV projections for TRN2, batch-sharded across
8 NeuronCores (one batch element per core).

Reference computation per batch element (S=2048, D=1024, fp32):
    Q = xq @ Wq + bq ; K = xk @ Wk + bk ; V = xv @ Wv + bv
    L = Q @ K^T                      # [S, S]
    out = (softmax(L, -1) * 1/sqrt(D)) @ V

v5: everything 2-byte on the PE (pipelined weight loads; fp32r matmuls
self-load serially on TRN2 so they are avoided entirely), plus the
associativity trick  L = Q K^T = xq (Wq Wk^T) xk^T  (exact here: bq = bk = 0
per the problem spec), which replaces the Q- and K-projections with ONE
projection by M = Wq Wk^T plus a cheap [D,D] weight-product:
  A-M:   load Wq,Wk (fp16 from host), PE-transpose both, M = Wq Wk^T via
         fp16 matmuls -> m_s resident fp16 (xq tiles stream in concurrently).
  A-tmp: tmp^T = M^T xq^T resident fp16 [D,S] (standard projection with m_s
         as the weights; fp16 x^T strips from PE transposes of GpSimd- er,
         DVE/ACT-converted fp16 x tiles).
  A-VK:  fused: per 512-row strip, PE-transpose xk tiles straight into the
         resident kt = xk^T [D,S] fp16 (no projection matmuls on the K path
         anymore) interleaved with the V projection (V = xv Wv, bf16
         resident) - the xk/xv DMA hides under the V matmuls.
  B:     logits L^T tiles = kt-tile stationary x tmp^T moving (fp16);
         exp -> bf16 U^T; AV bf16 with rowsum from 2-col matmuls reusing the
         AV stationaries; out = AV * (1/32)/rowsum + bv, DMA per half.

Precision: fp16 x/W/M/tmp^T/xk^T (exact fp16 products, fp32 accumulation)
+ bf16 exp/AV; measured 1.34e-2 vs the 2e-2 gate.

Scheduling refinements on top of the v5 structure: 4-buffer PSUM transpose
pool; A-VK issues the V-projection matmuls before the dead-end xk^T
transposes (PE filler while the next strip's xv DMA lands); phase-B output
stores alternate the two HWDGE queues. Cost-model timeline: 417us with the
PE 92.3% busy - the residual idle is startup DMA latency (the M product is
the first PE work and needs both weight matrices resident), strip-handoff
bubbles, and the final drain tail.
"""
import numpy as np
from contextlib import ExitStack

import concourse.bass as bass
import concourse.bacc as bacc
import concourse.tile as tile
import concourse.mybir as mybir
from concourse.bass_utils import run_bass_kernel_spmd

F32 = mybir.dt.float32
F32R = mybir.dt.float32r
F16 = mybir.dt.float16
BF16 = mybir.dt.bfloat16
AF = mybir.ActivationFunctionType

B, S, D = 8, 2048, 1024
NKT = D // 128          # 8 contraction tiles
NST = S // 128          # 16 s tiles
SCALE = 1.0 / 32.0      # 1/sqrt(D)

_CACHED = {}


def build(nrep=1, barrier=False):
    nc = bacc.Bacc("TRN2", target_bir_lowering=False, debug=False, num_devices=8)

    xq = nc.dram_tensor("xq", [S, D], F32R, kind="ExternalInput")
    xk = nc.dram_tensor("xk", [S, D], F32R, kind="ExternalInput")
    xv = nc.dram_tensor("xv", [S, D], F32R, kind="ExternalInput")
    wq = nc.dram_tensor("wq", [D, D], F16, kind="ExternalInput")
    wk = nc.dram_tensor("wk", [D, D], F16, kind="ExternalInput")
    wv = nc.dram_tensor("wv", [D, D], F16, kind="ExternalInput")
    bqd = nc.dram_tensor("bqd", [128, NKT], F32, kind="ExternalInput")  # bq.reshape(8,128).T
    bkd = nc.dram_tensor("bkd", [128, NKT], F32, kind="ExternalInput")
    bvd = nc.dram_tensor("bvd", [1, D], F32R, kind="ExternalInput")
    identd = nc.dram_tensor("identd", [128, 128], F16, kind="ExternalInput")
    ones1d = nc.dram_tensor("ones1d", [1, 128], F32R, kind="ExternalInput")
    onespd = nc.dram_tensor("onespd", [128, 2], BF16, kind="ExternalInput")

    out = nc.dram_tensor("out", [S, D], F32, kind="ExternalOutput")

    with tile.TileContext(nc) as tc, ExitStack() as ctx:
        # ---------------- persistent pools ----------------
        cpool = ctx.enter_context(tc.tile_pool(name="const", bufs=1))
        qtp = ctx.enter_context(tc.tile_pool(name="qtr", bufs=1))
        ktp = ctx.enter_context(tc.tile_pool(name="ktr", bufs=1))
        vsp = ctx.enter_context(tc.tile_pool(name="vres", bufs=1))
        pp = ctx.enter_context(tc.tile_pool(name="pp", bufs=2, space="PSUM"))

        ident = cpool.tile([128, 128], F16, tag="ident")
        bqs = cpool.tile([128, NKT], F32, tag="bqs")
        ones1 = cpool.tile([1, 128], F32R, tag="ones1")
        onesp = cpool.tile([128, 2], BF16, tag="onesp")
        bvb = cpool.tile([128, D], F32, tag="bvb")
        nc.sync.dma_start(ident[:], identd.ap())
        nc.gpsimd.dma_start(bqs[:], bqd.ap())
        nc.gpsimd.dma_start(ones1[:], ones1d.ap())
        nc.gpsimd.dma_start(onesp[:], onespd.ap())

        def broadcast_bv(bvctx):
            # bvb = ones1.T @ bvs via a K=1 matmul (issued at phase-B entry so
            # it does not block the PE queue head at startup)
            bvsp = bvctx.enter_context(tc.tile_pool(name="bvsp", bufs=1))
            bvs = bvsp.tile([1, D], F32R, tag="bvs")
            nc.gpsimd.dma_start(bvs[:], bvd.ap())
            for h in range(2):
                bps = pp.tile([128, 1024], F32, tag="pp")
                nc.tensor.matmul(bps[:, 0:512], ones1[:], bvs[:, h * 512:(h + 1) * 512],
                                 start=True, stop=True)
                nc.scalar.copy(bvb[:, h * 512:(h + 1) * 512], bps[:, 0:512])

        # ---------------- shared helpers ----------------
        def load_x_strip(xpool, xhpool, x_dram, j, n_stiles, alternate=False):
            """DMA x rows [j*128*n ..) as fp32 on the sync queue (phase-first
            strips alternate both queues), convert to fp16 on DVE/ACT."""
            xhs = []
            for st in range(n_stiles):
                xl = xpool.tile([128, D], F32R, tag="xl")
                dma = nc.scalar if (alternate and st % 2 == 1) else nc.sync
                dma.dma_start(
                    xl[:], x_dram.ap()[(j * n_stiles + st) * 128:(j * n_stiles + st + 1) * 128, :])
                xh = xhpool.tile([128, D], F16, tag="xh")
                if st % 2 == 0:
                    nc.scalar.copy(xh[:], xl[:])
                else:
                    nc.vector.tensor_copy(xh[:], xl[:])
                xhs.append(xh)
            return xhs

        def transpose_tiles(tp, dst, s_total, s_off, xhs):
            """PE-transpose fp16 tiles [128, D] and scatter into dst with
            k-major layout dst[:, k*s_total + s_off + st*128 + c]."""
            for st, xh in enumerate(xhs):
                tpt = tp.tile([128, NKT * 128], F16, tag="tp")
                for k in range(NKT):
                    nc.tensor.transpose(tpt[:, k * 128:(k + 1) * 128],
                                        xh[:, k * 128:(k + 1) * 128], ident[:])
                d3 = dst.rearrange("p (k s) -> p k s", s=s_total)
                off = s_off + st * 128
                if st % 2 == 0:
                    nc.vector.tensor_copy(d3[:, :, off:off + 128], tpt[:])
                else:
                    nc.scalar.copy(d3[:, :, off:off + 128], tpt[:])

        for _rep in range(nrep):
          if _rep and barrier:
              tc.strict_bb_all_engine_barrier()
          with ExitStack() as rctx:
            qt = qtp.tile([128, NKT * S], F16, tag="qt")     # tmp^T resident fp16
            kt = ktp.tile([128, NKT * S], F16, tag="kt")     # xk^T resident fp16
            vs = vsp.tile([128, NST * D], BF16, tag="vs")    # V resident bf16

            with ExitStack() as actx:
              tp = actx.enter_context(tc.tile_pool(name="tp", bufs=4, space="PSUM"))
              xpool = actx.enter_context(tc.tile_pool(name="xpool", bufs=6))
              xhpool = actx.enter_context(tc.tile_pool(name="xhpool", bufs=10))
              mpool = actx.enter_context(tc.tile_pool(name="mpool", bufs=1))

              # ---- A-M: M = Wq Wk^T resident fp16 [d_in-major, 1024] ----
              with nc.named_scope("phase_am"), ExitStack() as mctx:
                  wlp = mctx.enter_context(tc.tile_pool(name="wlp", bufs=3))
                  wtp = mctx.enter_context(tc.tile_pool(name="wtp", bufs=1))
                  wqT = wtp.tile([128, NKT * D], F16, tag="wqT")  # Wq^T, e-major
                  wkT = wtp.tile([128, NKT * D], F16, tag="wkT")
                  for w_dram, wT in ((wq, wqT), (wk, wkT)):
                      for k in range(NKT):
                          wl = wlp.tile([128, D], F16, tag="wl")
                          dma = nc.sync if k % 2 == 0 else nc.scalar
                          dma.dma_start(wl[:], w_dram.ap()[k * 128:(k + 1) * 128, :])
                          # transpose row-block k into wT columns k*128..
                          tpt = tp.tile([128, NKT * 128], F16, tag="tp")
                          for m in range(NKT):
                              nc.tensor.transpose(tpt[:, m * 128:(m + 1) * 128],
                                                  wl[:, m * 128:(m + 1) * 128], ident[:])
                          d3 = wT.rearrange("p (m s) -> p m s", s=D)
                          if k % 2 == 0:
                              nc.vector.tensor_copy(
                                  d3[:, :, k * 128:(k + 1) * 128], tpt[:])
                          else:
                              nc.scalar.copy(
                                  d3[:, :, k * 128:(k + 1) * 128], tpt[:])
                  # start streaming xq while M computes
                  xls_q = load_x_strip(xpool, xhpool, xq, 0, 8)
                  m_s = mpool.tile([128, NKT * D], F16, tag="m_s")
                  for i in range(NKT):          # d_in tiles of M
                      ppt = pp.tile([128, 1024], F32, tag="pp")
                      for e in range(NKT):
                          for h in range(2):
                              nc.tensor.matmul(
                                  ppt[:, h * 512:(h + 1) * 512],
                                  wqT[:, e * D + i * 128:e * D + (i + 1) * 128],
                                  wkT[:, e * D + h * 512:e * D + (h + 1) * 512],
                                  start=(e == 0), stop=(e == NKT - 1))
                      nc.scalar.copy(m_s[:, i * D:(i + 1) * D], ppt[:])

              # ---- A-tmp: tmp^T = M^T xq^T resident fp16 (M as weights) ----
              with nc.named_scope("phase_atmp"), ExitStack() as tctx:
                  xtpool = tctx.enter_context(tc.tile_pool(name="xtpool", bufs=1))
                  xls_kv = None
                  for j in range(2):
                      xls = xls_q if j == 0 else load_x_strip(xpool, xhpool, xq, j, 8)
                      xt = xtpool.tile([128, NKT * 1024], F16, tag="xt")
                      transpose_tiles(tp, xt, 1024, 0, xls)
                      if j == 1:
                          xls_kv = (load_x_strip(xpool, xhpool, xk, 0, 4, alternate=True),
                                    load_x_strip(xpool, xhpool, xv, 0, 4))
                      for m in range(NKT):
                          ppt = pp.tile([128, 1024], F32, tag="pp")
                          for k in range(NKT):
                              for h in range(2):
                                  nc.tensor.matmul(
                                      ppt[:, h * 512:(h + 1) * 512],
                                      m_s[:, k * D + m * 128:k * D + (m + 1) * 128],
                                      xt[:, k * 1024 + h * 512:k * 1024 + (h + 1) * 512],
                                      start=(k == 0), stop=(k == NKT - 1))
                          nc.scalar.activation(
                              qt[:, m * S + j * 1024:m * S + (j + 1) * 1024],
                              ppt[:], AF.Identity, bias=bqs[:, m:m + 1])

              # ---- A-VK fused: kt = xk^T (transposes only) + V = xv Wv ----
              with nc.named_scope("phase_avk"), ExitStack() as vctx:
                wpool = vctx.enter_context(tc.tile_pool(name="wpool", bufs=1))
                xtpool = vctx.enter_context(tc.tile_pool(name="xtpool", bufs=1))
                w_s = wpool.tile([128, NKT * D], F16, tag="w")
                for k in range(NKT):
                    nc.scalar.dma_start(w_s[:, k * D:(k + 1) * D],
                                        wv.ap()[k * 128:(k + 1) * 128, :])
                for j in range(4):
                    if j == 0:
                        xls_k, xls_v = xls_kv
                    else:
                        xls_k = load_x_strip(xpool, xhpool, xk, j, 4, alternate=True)
                        xls_v = load_x_strip(xpool, xhpool, xv, j, 4)
                    # V projection strip first (xk transposes are dead-end
                    # writes into kt - issue them after the V matmuls so they
                    # fill PE gaps while the next strip's xv tiles arrive)
                    xt = xtpool.tile([128, NKT * 512], F16, tag="xt")
                    transpose_tiles(tp, xt, 512, 0, xls_v)
                    for m in range(4):          # s tiles within strip
                        sg = j * 4 + m
                        ppt = pp.tile([128, 1024], F32, tag="pp")
                        for k in range(NKT):
                            for h in range(2):
                                nc.tensor.matmul(
                                    ppt[:, h * 512:(h + 1) * 512],
                                    xt[:, k * 512 + m * 128:k * 512 + (m + 1) * 128],
                                    w_s[:, k * D + h * 512:k * D + (h + 1) * 512],
                                    start=(k == 0), stop=(k == NKT - 1))
                        nc.scalar.copy(vs[:, sg * D:(sg + 1) * D], ppt[:])
                    transpose_tiles(tp, kt, S, j * 512, xls_k)

            # ---------------- phase B: attention (transposed logits) ----------------
            with ExitStack() as bctx, nc.named_scope("phase_b"):
              op = bctx.enter_context(tc.tile_pool(name="op", bufs=2, space="PSUM"))
              utp = bctx.enter_context(tc.tile_pool(name="utp", bufs=2))
              osp = bctx.enter_context(tc.tile_pool(name="osp", bufs=2))
              rsp = bctx.enter_context(tc.tile_pool(name="rsp", bufs=2))
              rsps = bctx.enter_context(tc.tile_pool(name="rsps", bufs=2, space="PSUM"))

              if _rep == 0:
                  broadcast_bv(bctx)

              for j in range(2):                  # q strips of 1024
                # L^T tiles + exp -> U^T strip [S, 1024] (bf16)
                ut = utp.tile([128, NST * 1024], BF16, tag="ut")
                for t in range(NST):
                    lpt = pp.tile([128, 1024], F32, tag="pp")
                    for k in range(NKT):
                        for h in range(2):
                            nc.tensor.matmul(
                                lpt[:, h * 512:(h + 1) * 512],
                                kt[:, k * S + t * 128:k * S + (t + 1) * 128],
                                qt[:, k * S + j * 1024 + h * 512:k * S + j * 1024 + (h + 1) * 512],
                                start=(k == 0), stop=(k == NKT - 1))
                    nc.scalar.activation(ut[:, t * 1024:(t + 1) * 1024],
                                         lpt[:], AF.Exp)

                for m in range(8):              # q tiles of 128 within strip
                    sq = j * 8 + m
                    # rowsum via 2-col matmuls sharing the AV stationaries
                    rs = rsps.tile([128, 2], F32, tag="rs")
                    rct = rsp.tile([128, 1], F32, tag="rct")
                    os_t = osp.tile([128, D], F32, tag="os")
                    for h in range(2):
                        opt = op.tile([128, 512], F32, tag="av")
                        for t in range(NST):
                            st_ap = ut[:, t * 1024 + m * 128:t * 1024 + (m + 1) * 128]
                            nc.tensor.matmul(
                                opt[:],
                                st_ap,
                                vs[:, t * D + h * 512:t * D + (h + 1) * 512],
                                start=(t == 0), stop=(t == NST - 1))
                            if h == 0:
                                nc.tensor.matmul(
                                    rs[:], st_ap, onesp[:],
                                    start=(t == 0), stop=(t == NST - 1))
                        if h == 0:
                            nc.vector.reciprocal(rct[:], rs[:, 0:1])
                            nc.vector.tensor_scalar_mul(rct[:], rct[:], SCALE)
                        nc.vector.tensor_scalar_mul(
                            os_t[:, h * 512:(h + 1) * 512], opt[:], rct[:])
                        nc.vector.tensor_add(
                            os_t[:, h * 512:(h + 1) * 512],
                            os_t[:, h * 512:(h + 1) * 512],
                            bvb[:, h * 512:(h + 1) * 512])
                        oq = nc.sync if h == 0 else nc.scalar
                        oq.dma_start(
                            out.ap()[sq * 128:(sq + 1) * 128, h * 512:(h + 1) * 512],
                            os_t[:, h * 512:(h + 1) * 512])

    nc.compile()
    return nc


def _get_nc():
    if "nc" not in _CACHED:
        _CACHED["nc"] = build()
    return _CACHED["nc"]


def _bf16_ones(shape):
    import ml_dtypes
    return np.ones(shape, ml_dtypes.bfloat16)


def make_in_maps(q, k, v, Wq, bq, Wk, bk, Wv, bv):
    q = np.ascontiguousarray(q, np.float32)
    k = np.ascontiguousarray(k, np.float32)
    v = np.ascontiguousarray(v, np.float32)
    consts = {
        "wq": np.ascontiguousarray(np.asarray(Wq, np.float32).astype(np.float16)),
        "wk": np.ascontiguousarray(np.asarray(Wk, np.float32).astype(np.float16)),
        "wv": np.ascontiguousarray(np.asarray(Wv, np.float32).astype(np.float16)),
        "bqd": np.ascontiguousarray(np.asarray(bq, np.float32).reshape(NKT, 128).T),
        "bkd": np.ascontiguousarray(np.asarray(bk, np.float32).reshape(NKT, 128).T),
        "bvd": np.asarray(bv, np.float32).reshape(1, D).copy(),
        "identd": np.eye(128, dtype=np.float16),
        "ones1d": np.ones((1, 128), np.float32),
        "onespd": _bf16_ones((128, 2)),
    }
    return [dict(consts, xq=q[c], xk=k[c], xv=v[c]) for c in range(B)]


def kernel(q, k, v, Wq, bq, Wk, bk, Wv, bv, _trace=False, _trace_kwargs=None):
    in_maps = make_in_maps(q, k, v, Wq, bq, Wk, bk, Wv, bv)
    nc = _get_nc()
    res = run_bass_kernel_spmd(nc, in_maps, core_ids=list(range(B)),
                               trace=_trace, **(_trace_kwargs or {}))
    out = np.stack([res.results[c]["out"] for c in range(B)])
    if _trace:
        kernel.last_results = res
    return out
